# revision 1
# baseline (speedup 1.0000x reference)
"""T5-style multi-head attention on 8 Trainium2 NeuronCores.

Problem: B=2, S=2048, D=1024, H=16 heads of 64; T5 relative-position bias
(32 buckets, max_distance=128), key mask, softmax, context.

Sharding: data-parallel over B (2) x tensor-parallel over head-groups of 4
(4 groups) = 8 cores.  Each core computes Q/K/V projections for its batch
and its 4 heads, then full attention for those heads.

Device algorithm (per core), all matmuls in float32r (full PE rate, ~1e-4):
  phase 1: stream X^T tiles from DRAM; V = X W_v in [s,d] layout,
           Q^T/K^T = (X W)^T in [d,s] layout (d on partitions).
  phase 2: per (head, 1024-wide q chunk, 128-wide k block):
           scoresT[k,q] = K^T.T Q^T   (contraction d=64)
           expS = exp(scoresT + c_maj)  on ACT (c_maj = saturated-bucket bias)
           band fix: expS *= exp(g(k-q) - c_maj) on the <=384-wide diagonal
           band (Toeplitz => one small [128,384] table per head/side);
           minority saturated side: expS *= exp(c_min - c_maj) (tensor-scalar)
           ctxT[d,q] += V_ext[k,d|1].T expS  (ones column => row 64 = softmax
           denominators, free)
  tail:    recip of denominators, broadcast, scale, DMA out ctxT per head.

The relative-position bucket table depends only on shapes; it and the
exp() of the bias table entries are precomputed host-side (tiny), shipped
as small tensors so a single SPMD program serves all 8 cores.
"""

import numpy as np

import concourse.bacc as bacc
import concourse.tile as tile
from concourse import mybir
from concourse.bass_utils import run_bass_kernel_spmd

# problem dims (hardcoded per contract)
B = 2
S = 2048
DM = 1024
H = 16
HD = 64
NB = 32
MAXD = 128

HPC = 4          # heads per core
NCORES = 8
NDT = DM // 128  # 8 contraction tiles
NKB = S // 128   # 16 k blocks
NQ2 = 2          # q chunks of 1024
QW = 1024        # q chunk width
EBW = 384        # band table width

F32 = mybir.dt.float32
F32R = mybir.dt.float32r
BF16 = mybir.dt.bfloat16

# bf16 projections halve the dominant input-DMA volume (X tensors);
# attention itself stays float32r
PROJ_BF16 = False
XDT = BF16 if PROJ_BF16 else F32R


def _rel_buckets():
    """T5 bidirectional bucket for rel = k - q in [-(S-1), S-1], fp32 math."""
    rel = np.arange(-(S - 1), S, dtype=np.int64)
    nb = NB // 2
    ret = (rel > 0).astype(np.int64) * nb
    rp = np.abs(rel)
    max_exact = nb // 2
    is_small = rp < max_exact
    rp_f = np.maximum(rp, 1).astype(np.float32)
    val = np.log(rp_f / np.float32(max_exact)) / np.float32(
        np.log(MAXD / max_exact)
    ) * np.float32(nb - max_exact)
    # XLA CPU f32->s32 convert rounds to nearest (cvtps2dq), not truncates
    val_large = max_exact + np.rint(val).astype(np.int32)
    val_large = np.minimum(val_large, nb - 1)
    return (ret + np.where(is_small, rp, val_large)).astype(np.int64)  # [2S-1]


def _band_bounds(kb):
    """Columns [a,b) of the non-saturated diagonal band for k block kb."""
    a = max(0, (kb - 1) * 128)
    b = min(S, (kb + 2) * 128)
    return a, b


def _maj_side(kb, q2):
    """Majority saturated side for (k block, q chunk): 0 -> bucket31 (q<a),
    1 -> bucket15 (q>=b)."""
    qlo, qhi = q2 * QW, (q2 + 1) * QW
    a, b = _band_bounds(kb)
    len31 = max(0, min(qhi, a) - qlo)
    len15 = max(0, qhi - max(qlo, b))
    return 0 if len31 >= len15 else 1


DEBUG_DUMPS = False


def build_program(use_mask, reps=1, ablate=()):
    nc = bacc.Bacc("TRN2", target_bir_lowering=False, debug=False,
                   num_devices=NCORES)

    xv = nc.dram_tensor("xv", [DM, S], XDT, kind="ExternalInput").ap()
    xq = nc.dram_tensor("xq", [DM, S], XDT, kind="ExternalInput").ap()
    xk = nc.dram_tensor("xk", [DM, S], XDT, kind="ExternalInput").ap()
    wq = nc.dram_tensor("wq", [DM, HPC * HD], XDT, kind="ExternalInput").ap()
    wk = nc.dram_tensor("wk", [DM, HPC * HD], XDT, kind="ExternalInput").ap()
    wv = nc.dram_tensor("wv", [DM, HPC * HD], XDT, kind="ExternalInput").ap()
    # band tables exp(g_h(rel) - c_maj): [side, head, 128, EBW]
    ebt = nc.dram_tensor("ebt", [2, HPC, 128, EBW], F32R,
                         kind="ExternalInput").ap()
    # per-(side, head): exp bias constant c_maj and minority ratio
    # cvals[0, side, h] = c_maj ; cvals[1, side, h] = exp(c_min - c_maj)
    cvals = nc.dram_tensor("cvals", [128, 2, 2, HPC], F32,
                           kind="ExternalInput").ap()
    vones = nc.dram_tensor("vones", [128, HPC * NKB], F32R,
                           kind="ExternalInput").ap()
    if use_mask:
        # additive mask term -1e4*(1-mask) laid out [128, NKB]
        mvals = nc.dram_tensor("mvals", [128, NKB], F32,
                               kind="ExternalInput").ap()
    outp = nc.dram_tensor("out", [HPC, HD, S], F32, kind="ExternalOutput").ap()
    dbg = {}
    if DEBUG_DUMPS:
        dbg["stg"] = nc.dram_tensor("d_stg", [HD + 1, QW], F32,
                                    kind="ExternalOutput").ap()
        dbg["rp"] = nc.dram_tensor("d_rp", [1, QW], F32,
                                   kind="ExternalOutput").ap()
        dbg["bc"] = nc.dram_tensor("d_bc", [HD, QW], F32,
                                   kind="ExternalOutput").ap()
        dbg["ot"] = nc.dram_tensor("d_ot", [HD, QW], F32,
                                   kind="ExternalOutput").ap()
        dbg["es"] = nc.dram_tensor("d_es", [NKB, 128, QW], F32,
                                   kind="ExternalOutput").ap()

    import concourse.bass as bass

    with tile.TileContext(nc) as tc:
        with tc.tile_pool(name="const", bufs=1) as const, \
             tc.tile_pool(name="qkt", bufs=1) as qkt, \
             tc.tile_pool(name="stgp", bufs=3) as stgp, \
             tc.tile_pool(name="outp_sb", bufs=2) as outp_sb, \
             tc.tile_pool(name="tailp", bufs=2) as tailp:

            # ---- resident constants (loaded just-in-time; wv first so
            # the xv stream starts immediately) ----
            w_dram = {"wv": wv, "wq": wq, "wk": wk}
            w_sb = {}

            def load_w(nm):
                t = const.tile([128, NDT, HPC * HD], XDT, tag=nm, name=nm)
                nc.sync.dma_start(
                    out=t[:],
                    in_=w_dram[nm].rearrange("(dt p) f -> p dt f", p=128))
                w_sb[nm] = t

            load_w("wv")
            eb_sb = const.tile([128, 2, HPC, EBW], F32R, tag="eb", name="eb")
            # broadcast c values down all partitions
            cb = const.tile([128, 2, 2, HPC], F32, tag="cb", name="cb")
            nc.sync.dma_start(out=cb[:], in_=cvals[:])
            if use_mask:
                mk = const.tile([128, NKB], F32, tag="mk", name="mk")
                nc.sync.dma_start(out=mk[:], in_=mvals[:])

            # per-pair Q^T/K^T [128(2 heads x 64d), S] and V_ext
            qt = [qkt.tile([128, S], F32R, tag=f"qt{p}", name=f"qt{p}") for p in range(2)]
            kt = [qkt.tile([128, S], F32R, tag=f"kt{p}", name=f"kt{p}") for p in range(2)]
            # V_ext: [128(k in block), head, kblock, 65(d|1)]
            vx = qkt.tile([128, HPC, NKB, HD + 1], F32R, tag="vx", name="vx")
            nc.sync.dma_start(
                out=vx[:, :, :, 0:1],
                in_=vones.rearrange("p (h k one) -> p h k one", h=HPC, one=1))

            # ---- phase 1: projections ----
            for _rep in range(reps):
              with tc.tile_pool(name="pjps", bufs=8, space="PSUM") as pjps, \
                   tc.tile_pool(name="xs", bufs=3) as xs:
                  # V projection: out V[s,d].  A PSUM bank holds ONE
                  # accumulation group (start=True clears the whole bank), so
                  # each sb gets its own bank: two waves of 8, xv resident.
                  xv_sb = []
                  for dt in range(NDT):
                      xt = xs.tile([128, S], XDT, tag=f"xv{dt}", name="xv8",
                                   bufs=1)
                      for c in range(4):
                          nc.sync.dma_start(
                              out=xt[:, c * 512:(c + 1) * 512],
                              in_=xv[dt * 128:(dt + 1) * 128,
                                     c * 512:(c + 1) * 512])
                      xv_sb.append(xt)
                  for wave in range(2):
                      vps = [pjps.tile([128, 256], F32, tag=f"pjv{i}",
                                       name="pjv", bufs=1) for i in range(8)]
                      for dt in range(NDT):
                          for i in range(8):
                              sb = wave * 8 + i
                              nc.tensor.matmul(
                                  vps[i][:],
                                  lhsT=xv_sb[dt][:, sb * 128:(sb + 1) * 128],
                                  rhs=w_sb["wv"][:, dt, :],
                                  start=(dt == 0), stop=(dt == NDT - 1))
                      for i in range(8):
                          sb = wave * 8 + i
                          nc.vector.tensor_copy(
                              out=vx[:, :, sb, 1:HD + 1],
                              in_=vps[i].rearrange("p (h d) -> p h d", h=HPC))

                  # Q/K projections: out (XW)^T [f, s], dt-outer
                  for wname, dst in (("wq", qt), ("wk", kt)):
                      if wname not in w_sb:
                          load_w(wname)
                      ps = [pjps.tile([128, 512], F32, tag=f"pjv{i}",
                                      name="pj", bufs=1) for i in range(8)]
                      for dt in range(NDT):
                          xt = xs.tile([128, S], XDT, tag=f"xv{dt}", name="xqk",
                                       bufs=1)
                          src = {"wq": xq, "wk": xk}[wname]
                          for c in range(4):
                              nc.sync.dma_start(
                                  out=xt[:, c * 512:(c + 1) * 512],
                                  in_=src[dt * 128:(dt + 1) * 128,
                                          c * 512:(c + 1) * 512])
                          for fb in range(2):
                              for sc in range(4):
                                  nc.tensor.matmul(
                                      ps[fb * 4 + sc][:],
                                      lhsT=w_sb[wname][:, dt,
                                                       fb * 128:(fb + 1) * 128
                                                       ],
                                      rhs=xt[:, sc * 512:(sc + 1) * 512
                                             ],
                                      start=(dt == 0), stop=(dt == NDT - 1))
                      for fb in range(2):
                          for sc in range(4):
                              # split the psum drain across DVE and the
                              # otherwise-idle ACT engine to shorten the
                              # phase-1 -> phase-2 PSUM pool handoff
                              copy = (nc.vector.tensor_copy if sc % 2 == 0
                                      else nc.scalar.copy)
                              copy(
                                  out=dst[fb][:, sc * 512:(sc + 1) * 512],
                                  in_=ps[fb * 4 + sc][:])

              # ---- phase 2: attention ----
              if _rep == 0:
                  nc.sync.dma_start(
                      out=eb_sb[:], in_=ebt.rearrange("m h p w -> p m h w"))
              with tc.tile_pool(name="atps", bufs=1, space="PSUM") as atps, \
                   tc.tile_pool(name="esp", bufs=4) as esp:
                  for h in range(HPC):
                      pr, hl = h // 2, h % 2
                      for q2 in range(NQ2):
                          ctx = atps.tile([HD + 1, QW], F32, tag="ctx", name="ctx", bufs=1)
                          for kb in range(NKB):
                              sps = atps.tile([128, QW], F32, tag="s", name="s", bufs=3)
                              for hf in range(2):
                                  qsl = qt[pr][hl * 64:(hl + 1) * 64,
                                               q2 * QW + hf * 512:
                                               q2 * QW + (hf + 1) * 512]
                                  ksl = kt[pr][hl * 64:(hl + 1) * 64,
                                               kb * 128:(kb + 1) * 128]
                                  nc.tensor.matmul(
                                      sps[:, hf * 512:(hf + 1) * 512],
                                      lhsT=ksl,
                                      rhs=qsl,
                                      start=True, stop=True)
                              if use_mask:
                                  nc.vector.tensor_scalar_add(
                                      sps[:], sps[:], mk[:, kb:kb + 1])
                              mi = _maj_side(kb, q2)
                              es = esp.tile([128, QW], F32R, tag="es", name="es")
                              nc.scalar.activation(
                                  out=es[:], in_=sps[:],
                                  func=mybir.ActivationFunctionType.Exp,
                                  bias=cb[:, 0, mi, h:h + 1], scale=1.0)
                              # band fix on DVE
                              a, b = _band_bounds(kb)
                              qlo = q2 * QW
                              bs, be = max(qlo, a), min(qlo + QW, b)
                              if bs < be:
                                  w0 = bs - (kb - 1) * 128
                                  nc.vector.tensor_mul(
                                      es[:, bs - qlo:be - qlo],
                                      es[:, bs - qlo:be - qlo],
                                      eb_sb[:, mi, h, w0:w0 + (be - bs)])
                              # minority saturated side on GPSIMD
                              if mi == 0:
                                  ms, me = max(qlo, b), qlo + QW
                              else:
                                  ms, me = qlo, min(qlo + QW, a)
                              if ms < me:
                                  nc.gpsimd.tensor_scalar_mul(
                                      es[:, ms - qlo:me - qlo],
                                      es[:, ms - qlo:me - qlo],
                                      cb[:, 1, mi, h:h + 1])
                              if DEBUG_DUMPS and h == 0 and q2 == 0:
                                  nc.sync.dma_start(out=dbg["es"][kb],
                                                    in_=es[:].bitcast(F32))
                              for hf in range(2):
                                  nc.tensor.matmul(
                                      ctx[:, hf * 512:(hf + 1) * 512],
                                      lhsT=vx[:, h, kb, :],
                                      rhs=es[:, hf * 512:(hf + 1) * 512
                                             ],
                                      start=(kb == 0), stop=(kb == NKB - 1))
                          # evacuate psum; row 0 = softmax denominators
                          # (ones column is at V_ext index 0 so the denom
                          # lands on partition 0 for the reciprocal)
                          stg = stgp.tile([HD + 1, QW], F32, tag="stg", name="stg")
                          nc.vector.tensor_copy(out=stg[:], in_=ctx[:])
                          rp = tailp.tile([1, QW], F32, tag="rp", name="rp")
                          # single-pass approx (~51 ULP) — ample for the
                          # well-conditioned softmax denominators
                          nc.vector.reciprocal_approx_fast(
                              out=rp[:], in_=stg[0:1, :])
                          bcast = tailp.tile([HD + 1, QW], F32, tag="bc",
                                             name="bc")
                          nc.gpsimd.partition_broadcast(bcast[:], rp[:])
                          ot = outp_sb.tile([HD + 1, QW], F32, tag="ot",
                                            name="ot")
                          nc.vector.tensor_mul(ot[0:HD + 1, :],
                                               stg[0:HD + 1, :],
                                               bcast[0:HD + 1, :])
                          nc.sync.dma_start(
                              out=outp[h, :, q2 * QW:(q2 + 1) * QW],
                              in_=ot[1:HD + 1, :])
                          if DEBUG_DUMPS and h == 0 and q2 == 0:
                              nc.sync.dma_start(out=dbg["stg"], in_=stg[:])
                              nc.sync.dma_start(out=dbg["rp"], in_=rp[:])
                              nc.sync.dma_start(out=dbg["bc"], in_=bcast[:])
                              nc.sync.dma_start(out=dbg["ot"], in_=ot[:])

    nc.finalize()
    return nc


_PROG_CACHE = {}


def _get_program(use_mask):
    key = bool(use_mask)
    if key not in _PROG_CACHE:
        _PROG_CACHE[key] = build_program(key)
    return _PROG_CACHE[key]


def kernel(query, key, value, key_mask, Wq, Wk, Wv, bias_table):
    query = np.asarray(query, dtype=np.float32)
    key = np.asarray(key, dtype=np.float32)
    value = np.asarray(value, dtype=np.float32)
    key_mask = np.asarray(key_mask, dtype=np.float32)
    Wq = np.asarray(Wq, dtype=np.float32)
    Wk = np.asarray(Wk, dtype=np.float32)
    Wv = np.asarray(Wv, dtype=np.float32)
    bias_table = np.asarray(bias_table, dtype=np.float32)

    use_mask = not np.all(key_mask == 1.0)
    nc = _get_program(use_mask)

    buckets = _rel_buckets()  # [2S-1] for rel = k-q in [-(S-1), S-1]
    g = bias_table[buckets]   # [2S-1, H] bias as function of rel
    in_maps = []
    for core in range(NCORES):
        b, hg = core // 4, core % 4
        hsl = slice(hg * HPC * HD, (hg + 1) * HPC * HD)
        heads = np.arange(hg * HPC, (hg + 1) * HPC)
        c31 = bias_table[31, heads]  # rel >= +128
        c15 = bias_table[15, heads]  # rel <= -128
        cmaj = np.stack([c31, c15])               # [side, h]
        cmin = np.stack([c15, c31])
        # -32 keeps the unnormalized exps in a sane fp32 range (softmax is
        # shift-invariant; numerator and denominator scale together)
        cv = np.stack([cmaj - 32.0, np.exp(cmin - cmaj)]).astype(np.float32)
        # band tables: ebt[side, h, p, w] = exp(g_h(p - w + 128) - cmaj)
        p = np.arange(128)[:, None]
        w = np.arange(EBW)[None, :]
        rel = p - w + 128                          # in (-256, 256)
        gh = g[rel + (S - 1)][:, :, heads]         # [128, EBW, HPC]
        ebt_np = np.empty((2, HPC, 128, EBW), np.float32)
        for mi in range(2):
            ebt_np[mi] = np.exp(
                gh - cmaj[mi][None, None, :]).transpose(2, 0, 1)
        if PROJ_BF16:
            import ml_dtypes
            xdt = ml_dtypes.bfloat16
        else:
            xdt = np.float32
        im = {
            "xv": np.ascontiguousarray(value[b].T).astype(xdt),
            "xq": np.ascontiguousarray(query[b].T).astype(xdt),
            "xk": np.ascontiguousarray(key[b].T).astype(xdt),
            "wq": np.ascontiguousarray(Wq[:, hsl]).astype(xdt),
            "wk": np.ascontiguousarray(Wk[:, hsl]).astype(xdt),
            "wv": np.ascontiguousarray(Wv[:, hsl]).astype(xdt),
            "ebt": ebt_np,
            "cvals": np.broadcast_to(cv, (128,) + cv.shape).copy(),
            "vones": np.ones((128, HPC * NKB), np.float32),
        }
        if use_mask:
            madd = (-1e4 * (1.0 - key_mask[b])).astype(np.float32)
            im["mvals"] = np.ascontiguousarray(
                madd.reshape(NKB, 128).T)
        in_maps.append(im)

    res = run_bass_kernel_spmd(nc, in_maps, core_ids=list(range(NCORES)))
    out = np.empty((B, S, H * HD), np.float32)
    for core in range(NCORES):
        b, hg = core // 4, core % 4
        o = res.results[core]["out"]  # [HPC, HD, S]
        for h in range(HPC):
            out[b, :, (hg * HPC + h) * HD:(hg * HPC + h + 1) * HD] = o[h].T
    return out



# revision 10
# speedup vs baseline: 674.1270x; 674.1270x over previous
"""T5-style multi-head attention on 8 Trainium2 NeuronCores.

Problem: B=2, S=2048, D=1024, H=16 heads of 64; T5 relative-position bias
(32 buckets, max_distance=128), key mask, softmax, context.

Sharding: data-parallel over B (2) x tensor-parallel over head-groups of 4
(4 groups) = 8 cores.  Each core computes Q/K/V projections for its batch
and its 4 heads, then full attention for those heads.

v2: bf16 inputs/weights/activations (halves the input-DMA volume and
enables fast-weight-load), column-streamed projections software-pipelined
with the attention phase so the ACT engine (exp is the per-core compute
floor at ~126us) starts ~20us in instead of after all projections.

Device algorithm (per core), matmul cycles at 1 col/cycle in bf16:
  stream order: xk/xq/xv column-chunks of 1024; projections consume each
  chunk as it lands (Q/K as (X W)^T in [f, s] layout, V in [s, d] layout
  with a ones column for the softmax denominator).
  attention per (head, q2 chunk of 1024, k block of 128):
    scoresT[k,q] = K^T.T Q^T   (contraction d=64)
    expS = exp(scoresT + c_maj - 32) on ACT (c_maj = saturated-bucket bias)
    band fix (DVE) + minority saturated side (GPSIMD) as multiplicative
    corrections; ctxT[d|1,q] += V_ext.T expS accumulated over k blocks;
    row 0 of ctxT = softmax denominators (ones column rides along free).
  tail: reciprocal (DVE), partition broadcast (GPSIMD), scale, DMA out.

The first head's first-half k blocks are emitted before the second half
of the K/V streams arrive so ACT has work during the stream tail; its es
tiles for head h1 are held in SBUF until V lands.
"""

import numpy as np

import concourse.bacc as bacc
import concourse.tile as tile
from concourse import mybir
from concourse.bass_utils import run_bass_kernel_spmd

# problem dims (hardcoded per contract)
B = 2
S = 2048
DM = 1024
H = 16
HD = 64
NB = 32
MAXD = 128

HPC = 4          # heads per core
NCORES = 8
NDT = DM // 128  # 8 contraction tiles
NKB = S // 128   # 16 k blocks
NQ2 = 2          # q chunks of 1024
QW = 1024        # q chunk width
EBW = 384        # band table width

F32 = mybir.dt.float32
F32R = mybir.dt.float32r
BF16 = mybir.dt.bfloat16
F16 = mybir.dt.float16


def _rel_buckets():
    """T5 bidirectional bucket for rel = k - q in [-(S-1), S-1], fp32 math."""
    rel = np.arange(-(S - 1), S, dtype=np.int64)
    nb = NB // 2
    ret = (rel > 0).astype(np.int64) * nb
    rp = np.abs(rel)
    max_exact = nb // 2
    is_small = rp < max_exact
    rp_f = np.maximum(rp, 1).astype(np.float32)
    val = np.log(rp_f / np.float32(max_exact)) / np.float32(
        np.log(MAXD / max_exact)
    ) * np.float32(nb - max_exact)
    # XLA CPU f32->s32 convert rounds to nearest (cvtps2dq), not truncates
    val_large = max_exact + np.rint(val).astype(np.int32)
    val_large = np.minimum(val_large, nb - 1)
    return (ret + np.where(is_small, rp, val_large)).astype(np.int64)  # [2S-1]


def _band_bounds(kb):
    """Columns [a,b) of the non-saturated diagonal band for k block kb."""
    a = max(0, (kb - 1) * 128)
    b = min(S, (kb + 2) * 128)
    return a, b


def _maj_side(kb, q2):
    """Majority saturated side for (k block, q chunk): 0 -> bucket31 (q<a),
    1 -> bucket15 (q>=b)."""
    qlo, qhi = q2 * QW, (q2 + 1) * QW
    a, b = _band_bounds(kb)
    len31 = max(0, min(qhi, a) - qlo)
    len15 = max(0, qhi - max(qlo, b))
    return 0 if len31 >= len15 else 1


def build_program(use_mask, reps=1):
    nc = bacc.Bacc("TRN2", target_bir_lowering=False, debug=False,
                   num_devices=NCORES)

    xv = nc.dram_tensor("xv", [DM, S], F16, kind="ExternalInput").ap()
    xq = nc.dram_tensor("xq", [DM, S], F16, kind="ExternalInput").ap()
    xk = nc.dram_tensor("xk", [DM, S], F16, kind="ExternalInput").ap()
    wq = nc.dram_tensor("wq", [DM, HPC * HD], F16, kind="ExternalInput").ap()
    wk = nc.dram_tensor("wk", [DM, HPC * HD], F16, kind="ExternalInput").ap()
    wv = nc.dram_tensor("wv", [DM, HPC * HD], F16, kind="ExternalInput").ap()
    # band tables exp(g_h(rel) - c_maj): [side, head, 128, EBW]
    ebt = nc.dram_tensor("ebt", [2, HPC, 128, EBW], BF16,
                         kind="ExternalInput").ap()
    # per-(side, head): exp bias constant c_maj and minority ratio
    # cvals[0, side, h] = c_maj - 32 ; cvals[1, side, h] = exp(c_min - c_maj)
    cvals = nc.dram_tensor("cvals", [128, 2, 2, HPC], F32,
                           kind="ExternalInput").ap()
    vones = nc.dram_tensor("vones", [128, HPC * NKB], BF16,
                           kind="ExternalInput").ap()
    if use_mask:
        # additive mask term -1e4*(1-mask) laid out [128, NKB]
        mvals = nc.dram_tensor("mvals", [128, NKB], F32,
                               kind="ExternalInput").ap()
    # row 0 = softmax denominators, rows 1..64 = unnormalized context;
    # the division happens host-side during unsharding
    outp = nc.dram_tensor("out", [HPC, HD + 1, S], F32,
                          kind="ExternalOutput").ap()

    with tile.TileContext(nc) as tc:
        with tc.tile_pool(name="const", bufs=1) as const, \
             tc.tile_pool(name="qkt", bufs=1) as qkt, \
             tc.tile_pool(name="xs", bufs=3) as xs, \
             tc.tile_pool(name="esp", bufs=14) as esp, \
             tc.tile_pool(name="stgp", bufs=3) as stgp:

            # ---- resident constants ----
            w_sb = {}
            for nm, src in (("wk", wk), ("wq", wq), ("wv", wv)):
                t = const.tile([128, NDT, HPC * HD], F16, tag=nm, name=nm)
                nc.sync.dma_start(
                    out=t[:], in_=src.rearrange("(dt p) f -> p dt f", p=128))
                w_sb[nm] = t
            cb = const.tile([128, 2, 2, HPC], F32, tag="cb", name="cb")
            nc.sync.dma_start(out=cb[:], in_=cvals[:])
            eb_sb = const.tile([128, 2, HPC, EBW], BF16, tag="eb", name="eb")
            nc.sync.dma_start(out=eb_sb[:],
                              in_=ebt.rearrange("m h p w -> p m h w"))
            if use_mask:
                mk = const.tile([128, NKB], F32, tag="mk", name="mk")
                nc.sync.dma_start(out=mk[:], in_=mvals[:])

            # Q^T/K^T per pair: [128(2 heads x 64d), S] bf16
            qt = [qkt.tile([128, S], F16, tag=f"qt{p}", name=f"qt{p}")
                  for p in range(2)]
            kt = [qkt.tile([128, S], F16, tag=f"kt{p}", name=f"kt{p}")
                  for p in range(2)]
            # V_ext: [128(k in block), head, kblock, 65(1|d)]
            vx = qkt.tile([128, HPC, NKB, HD + 1], BF16, tag="vx", name="vx")
            nc.sync.dma_start(
                out=vx[:, :, :, 0:1],
                in_=vones.rearrange("p (h k one) -> p h k one", h=HPC, one=1))

            for _rep in range(reps):
              # psum pools: sps 2x[128,1024] (4 banks) + ctx 1x[65,1024]
              # (2 banks) + pp 1x[128,1024] (2 banks) = 8 banks
              with tc.tile_pool(name="pp", bufs=1, space="PSUM") as pp, \
                   tc.tile_pool(name="spsp", bufs=2, space="PSUM") as spsp, \
                   tc.tile_pool(name="ctxp", bufs=1, space="PSUM") as ctxp:

                def stream_chunk(src, tag, c2):
                    """DMA one [dm, 1024]-column chunk as 8 dt tiles."""
                    tiles = []
                    for dt in range(NDT):
                        t = xs.tile([128, QW], F16, tag=f"x{dt}",
                                    name=f"x{tag}")
                        nc.sync.dma_start(
                            out=t[:],
                            in_=src[dt * 128:(dt + 1) * 128,
                                    c2 * QW:(c2 + 1) * QW])
                        tiles.append(t)
                    return tiles

                def qk_proj(wname, dst, xt, c2, drain):
                    """(X W)^T for one 1024-col chunk; fb-sequential."""
                    for fb in range(2):
                        ps = pp.tile([128, QW], F32, tag="pp", name="pj")
                        for dt in range(NDT):
                            for sc in range(2):
                                nc.tensor.matmul(
                                    ps[:, sc * 512:(sc + 1) * 512],
                                    lhsT=w_sb[wname][:, dt,
                                                     fb * 128:(fb + 1) * 128],
                                    rhs=xt[dt][:, sc * 512:(sc + 1) * 512],
                                    start=(dt == 0), stop=(dt == NDT - 1))
                        drain(out=dst[fb][:, c2 * QW:(c2 + 1) * QW],
                              in_=ps[:])

                def v_proj(xt, c2, drain):
                    """V[s,d] for 8 s-blocks; 2 blocks per psum tile
                    (cols 0-255 of each bank)."""
                    for i in range(4):
                        ps = pp.tile([128, QW], F32, tag="pp", name="pv")
                        for dt in range(NDT):
                            for j in range(2):
                                sb = i * 2 + j
                                nc.tensor.matmul(
                                    ps[:, j * 512:j * 512 + 256],
                                    lhsT=xt[dt][:, sb * 128:(sb + 1) * 128],
                                    rhs=w_sb["wv"][:, dt, :],
                                    start=(dt == 0), stop=(dt == NDT - 1))
                        for j in range(2):
                            sb = c2 * 8 + i * 2 + j
                            drain(
                                out=vx[:, :, sb, 1:HD + 1],
                                in_=ps[:, j * 512:j * 512 + 256].rearrange(
                                    "p (h d) -> p h d", h=HPC))

                def attn_scores(h, q2, kb):
                    """scores + exp + region fixes -> es tile (SBUF bf16)."""
                    pr, hl = h // 2, h % 2
                    sps = spsp.tile([128, QW], F32, tag="s", name="s")
                    for hf in range(2):
                        qsl = qt[pr][hl * 64:(hl + 1) * 64,
                                     q2 * QW + hf * 512:
                                     q2 * QW + (hf + 1) * 512]
                        ksl = kt[pr][hl * 64:(hl + 1) * 64,
                                     kb * 128:(kb + 1) * 128]
                        nc.tensor.matmul(
                            sps[:, hf * 512:(hf + 1) * 512],
                            lhsT=ksl, rhs=qsl, start=True, stop=True)
                    if use_mask:
                        nc.vector.tensor_scalar_add(
                            sps[:], sps[:], mk[:, kb:kb + 1])
                    mi = _maj_side(kb, q2)
                    es = esp.tile([128, QW], BF16, tag="es", name="es")
                    nc.scalar.activation(
                        out=es[:], in_=sps[:],
                        func=mybir.ActivationFunctionType.Exp,
                        bias=cb[:, 0, mi, h:h + 1], scale=1.0)
                    # band fix on DVE
                    a, b = _band_bounds(kb)
                    qlo = q2 * QW
                    bs, be = max(qlo, a), min(qlo + QW, b)
                    if bs < be:
                        w0 = bs - (kb - 1) * 128
                        nc.vector.tensor_mul(
                            es[:, bs - qlo:be - qlo],
                            es[:, bs - qlo:be - qlo],
                            eb_sb[:, mi, h, w0:w0 + (be - bs)])
                    # minority saturated side on GPSIMD
                    if mi == 0:
                        ms, me = max(qlo, b), qlo + QW
                    else:
                        ms, me = qlo, min(qlo + QW, a)
                    if ms < me:
                        nc.gpsimd.tensor_scalar_mul(
                            es[:, ms - qlo:me - qlo],
                            es[:, ms - qlo:me - qlo],
                            cb[:, 1, mi, h:h + 1])
                    return es

                def attn_ctx(ctx, h, kb, es):
                    for hf in range(2):
                        nc.tensor.matmul(
                            ctx[:, hf * 512:(hf + 1) * 512],
                            lhsT=vx[:, h, kb, :],
                            rhs=es[:, hf * 512:(hf + 1) * 512],
                            start=(kb == 0), stop=(kb == NKB - 1))

                def attn_tail(ctx, h, q2):
                    # evacuate psum; row 0 = softmax denominators
                    # (normalization happens host-side during unsharding)
                    stg = stgp.tile([HD + 1, QW], F32, tag="stg", name="stg")
                    nc.vector.tensor_copy(out=stg[:], in_=ctx[:])
                    nc.sync.dma_start(
                        out=outp[h, :, q2 * QW:(q2 + 1) * QW],
                        in_=stg[:])

                def attn_head(h, q2, kbs, ctx, es_pre=None):
                    for kb in kbs:
                        es = es_pre.pop(kb, None) if es_pre else None
                        if es is None:
                            es = attn_scores(h, q2, kb)
                        attn_ctx(ctx, h, kb, es)

                # ---- streaming + overlapped schedule ----
                dve_drain = nc.vector.tensor_copy
                act_drain = nc.scalar.copy

                xk0 = stream_chunk(xk, "k", 0)
                qk_proj("wk", kt, xk0, 0, act_drain)     # ACT idle early
                xq0 = stream_chunk(xq, "q", 0)
                qk_proj("wq", qt, xq0, 0, act_drain)
                xv0 = stream_chunk(xv, "v", 0)
                v_proj(xv0, 0, act_drain)

                # h0: first-half k blocks (kt/vx blocks 0-7 ready)
                ctx0 = ctxp.tile([HD + 1, QW], F32, tag="ctx", name="ctx")
                attn_head(0, 0, range(8), ctx0)
                # h1: scores only; es held in SBUF until vx 8-15 lands
                es_h1 = {kb: attn_scores(1, 0, kb) for kb in range(8)}

                # second halves of the K and V streams
                xk1 = stream_chunk(xk, "k", 1)
                qk_proj("wk", kt, xk1, 1, dve_drain)
                xv1 = stream_chunk(xv, "v", 1)
                v_proj(xv1, 1, dve_drain)

                # finish h0, then h1 (reusing held es), then h2/h3
                attn_head(0, 0, range(8, NKB), ctx0)
                attn_tail(ctx0, 0, 0)
                ctx1 = ctxp.tile([HD + 1, QW], F32, tag="ctx", name="ctx")
                attn_head(1, 0, range(NKB), ctx1, es_pre=es_h1)
                attn_tail(ctx1, 1, 0)
                for h in (2, 3):
                    ctx = ctxp.tile([HD + 1, QW], F32, tag="ctx", name="ctx")
                    attn_head(h, 0, range(NKB), ctx)
                    attn_tail(ctx, h, 0)

                # q2=1: needs the last input chunk
                xq1 = stream_chunk(xq, "q", 1)
                qk_proj("wq", qt, xq1, 1, dve_drain)
                for h in range(HPC):
                    ctx = ctxp.tile([HD + 1, QW], F32, tag="ctx", name="ctx")
                    attn_head(h, 1, range(NKB), ctx)
                    attn_tail(ctx, h, 1)

    nc.finalize()
    return nc


_PROG_CACHE = {}


def _get_program(use_mask):
    key = bool(use_mask)
    if key not in _PROG_CACHE:
        _PROG_CACHE[key] = build_program(key)
    return _PROG_CACHE[key]


def kernel(query, key, value, key_mask, Wq, Wk, Wv, bias_table):
    import ml_dtypes
    bf16 = ml_dtypes.bfloat16
    f16 = np.float16

    query = np.asarray(query, dtype=np.float32)
    key = np.asarray(key, dtype=np.float32)
    value = np.asarray(value, dtype=np.float32)
    key_mask = np.asarray(key_mask, dtype=np.float32)
    Wq = np.asarray(Wq, dtype=np.float32)
    Wk = np.asarray(Wk, dtype=np.float32)
    Wv = np.asarray(Wv, dtype=np.float32)
    bias_table = np.asarray(bias_table, dtype=np.float32)

    use_mask = not np.all(key_mask == 1.0)
    nc = _get_program(use_mask)

    buckets = _rel_buckets()  # [2S-1] for rel = k-q in [-(S-1), S-1]
    g = bias_table[buckets]   # [2S-1, H] bias as function of rel
    in_maps = []
    for core in range(NCORES):
        b, hg = core // 4, core % 4
        hsl = slice(hg * HPC * HD, (hg + 1) * HPC * HD)
        heads = np.arange(hg * HPC, (hg + 1) * HPC)
        c31 = bias_table[31, heads]  # rel >= +128
        c15 = bias_table[15, heads]  # rel <= -128
        cmaj = np.stack([c31, c15])               # [side, h]
        cmin = np.stack([c15, c31])
        # -32 keeps the unnormalized exps in a sane fp32 range (softmax is
        # shift-invariant; numerator and denominator scale together)
        cv = np.stack([cmaj - 32.0, np.exp(cmin - cmaj)]).astype(np.float32)
        # band tables: ebt[side, h, p, w] = exp(g_h(p - w + 128) - cmaj)
        p = np.arange(128)[:, None]
        w = np.arange(EBW)[None, :]
        rel = p - w + 128                          # in (-256, 256)
        gh = g[rel + (S - 1)][:, :, heads]         # [128, EBW, HPC]
        ebt_np = np.empty((2, HPC, 128, EBW), np.float32)
        for mi in range(2):
            ebt_np[mi] = np.exp(
                gh - cmaj[mi][None, None, :]).transpose(2, 0, 1)
        im = {
            "xv": np.ascontiguousarray(value[b].T).astype(f16),
            "xq": np.ascontiguousarray(query[b].T).astype(f16),
            "xk": np.ascontiguousarray(key[b].T).astype(f16),
            "wq": np.ascontiguousarray(Wq[:, hsl]).astype(f16),
            "wk": np.ascontiguousarray(Wk[:, hsl]).astype(f16),
            "wv": np.ascontiguousarray(Wv[:, hsl]).astype(f16),
            "ebt": ebt_np.astype(bf16),
            "cvals": np.broadcast_to(cv, (128,) + cv.shape).copy(),
            "vones": np.ones((128, HPC * NKB), bf16),
        }
        if use_mask:
            madd = (-1e4 * (1.0 - key_mask[b])).astype(np.float32)
            im["mvals"] = np.ascontiguousarray(madd.reshape(NKB, 128).T)
        in_maps.append(im)

    res = run_bass_kernel_spmd(nc, in_maps, core_ids=list(range(NCORES)))
    out = np.empty((B, S, H * HD), np.float32)
    for core in range(NCORES):
        b, hg = core // 4, core % 4
        o = res.results[core]["out"]  # [HPC, HD+1, S]; row 0 = denominators
        for h in range(HPC):
            out[b, :, (hg * HPC + h) * HD:(hg * HPC + h + 1) * HD] = \
                (o[h, 1:] / o[h, 0:1]).T
    return out


# revision 12
# speedup vs baseline: 866.1273x; 1.2848x over previous
"""T5-style multi-head attention on 8 Trainium2 NeuronCores.

Problem: B=2, S=2048, D=1024, H=16 heads of 64; T5 relative-position bias
(32 buckets, max_distance=128), key mask, softmax, context.

Sharding: data-parallel over B (2) x tensor-parallel over head-groups of 4
(4 groups) = 8 cores.  Each core computes Q/K/V projections for its batch
and its 4 heads, then full attention for those heads.

v2: bf16 inputs/weights/activations (halves the input-DMA volume and
enables fast-weight-load), column-streamed projections software-pipelined
with the attention phase so the ACT engine (exp is the per-core compute
floor at ~126us) starts ~20us in instead of after all projections.

Device algorithm (per core), matmul cycles at 1 col/cycle in bf16:
  stream order: xk/xq/xv column-chunks of 1024; projections consume each
  chunk as it lands (Q/K as (X W)^T in [f, s] layout, V in [s, d] layout
  with a ones column for the softmax denominator).
  attention per (head, q2 chunk of 1024, k block of 128):
    scoresT[k,q] = K^T.T Q^T   (contraction d=64)
    expS = exp(scoresT + c_maj - 32) on ACT (c_maj = saturated-bucket bias)
    band fix (DVE) + minority saturated side (GPSIMD) as multiplicative
    corrections; ctxT[d|1,q] += V_ext.T expS accumulated over k blocks;
    row 0 of ctxT = softmax denominators (ones column rides along free).
  tail: reciprocal (DVE), partition broadcast (GPSIMD), scale, DMA out.

The first head's first-half k blocks are emitted before the second half
of the K/V streams arrive so ACT has work during the stream tail; its es
tiles for head h1 are held in SBUF until V lands.
"""

import numpy as np

import concourse.bacc as bacc
import concourse.tile as tile
from concourse import mybir
from concourse.bass_utils import run_bass_kernel_spmd

# problem dims (hardcoded per contract)
B = 2
S = 2048
DM = 1024
H = 16
HD = 64
NB = 32
MAXD = 128

HPC = 4          # heads per core
NCORES = 8
NDT = DM // 128  # 8 contraction tiles
NKB = S // 128   # 16 k blocks
NQ2 = 4          # q chunks of 512
QW = 512         # q chunk width
CW = 1024        # input stream chunk width
EBW = 384        # band table width

F32 = mybir.dt.float32
F32R = mybir.dt.float32r
BF16 = mybir.dt.bfloat16
F16 = mybir.dt.float16


def _rel_buckets():
    """T5 bidirectional bucket for rel = k - q in [-(S-1), S-1], fp32 math."""
    rel = np.arange(-(S - 1), S, dtype=np.int64)
    nb = NB // 2
    ret = (rel > 0).astype(np.int64) * nb
    rp = np.abs(rel)
    max_exact = nb // 2
    is_small = rp < max_exact
    rp_f = np.maximum(rp, 1).astype(np.float32)
    val = np.log(rp_f / np.float32(max_exact)) / np.float32(
        np.log(MAXD / max_exact)
    ) * np.float32(nb - max_exact)
    # XLA CPU f32->s32 convert rounds to nearest (cvtps2dq), not truncates
    val_large = max_exact + np.rint(val).astype(np.int32)
    val_large = np.minimum(val_large, nb - 1)
    return (ret + np.where(is_small, rp, val_large)).astype(np.int64)  # [2S-1]


def _band_bounds(kb):
    """Columns [a,b) of the non-saturated diagonal band for k block kb."""
    a = max(0, (kb - 1) * 128)
    b = min(S, (kb + 2) * 128)
    return a, b


def _maj_side(kb, q2):
    """Majority saturated side for (k block, q chunk): 0 -> bucket31 (q<a),
    1 -> bucket15 (q>=b)."""
    qlo, qhi = q2 * QW, (q2 + 1) * QW
    a, b = _band_bounds(kb)
    len31 = max(0, min(qhi, a) - qlo)
    len15 = max(0, qhi - max(qlo, b))
    return 0 if len31 >= len15 else 1


def build_program(use_mask, reps=1):
    nc = bacc.Bacc("TRN2", target_bir_lowering=False, debug=False,
                   num_devices=NCORES)

    xv = nc.dram_tensor("xv", [DM, S], F16, kind="ExternalInput").ap()
    xq = nc.dram_tensor("xq", [DM, S], F16, kind="ExternalInput").ap()
    xk = nc.dram_tensor("xk", [DM, S], F16, kind="ExternalInput").ap()
    wq = nc.dram_tensor("wq", [DM, HPC * HD], F16, kind="ExternalInput").ap()
    wk = nc.dram_tensor("wk", [DM, HPC * HD], F16, kind="ExternalInput").ap()
    wv = nc.dram_tensor("wv", [DM, HPC * HD], F16, kind="ExternalInput").ap()
    # band tables exp(g_h(rel) - c_maj): [side, head, 128, EBW]
    ebt = nc.dram_tensor("ebt", [2, HPC, 128, EBW], BF16,
                         kind="ExternalInput").ap()
    # per-(side, head): exp bias constant c_maj and minority ratio
    # cvals[0, side, h] = c_maj - 32 ; cvals[1, side, h] = exp(c_min - c_maj)
    cvals = nc.dram_tensor("cvals", [128, 2, 2, HPC], F32,
                           kind="ExternalInput").ap()
    vones = nc.dram_tensor("vones", [128, HPC * NKB], BF16,
                           kind="ExternalInput").ap()
    if use_mask:
        # additive mask term -1e4*(1-mask) laid out [128, NKB]
        mvals = nc.dram_tensor("mvals", [128, NKB], F32,
                               kind="ExternalInput").ap()
    # row 0 = softmax denominators, rows 1..64 = unnormalized context;
    # the division happens host-side during unsharding
    outp = nc.dram_tensor("out", [HPC, HD + 1, S], F32,
                          kind="ExternalOutput").ap()

    with tile.TileContext(nc) as tc:
        with tc.tile_pool(name="const", bufs=1) as const, \
             tc.tile_pool(name="qkt", bufs=1) as qkt, \
             tc.tile_pool(name="xs", bufs=3) as xs, \
             tc.tile_pool(name="esp", bufs=24) as esp, \
             tc.tile_pool(name="stgp", bufs=3) as stgp:

            # ---- resident constants ----
            w_sb = {}
            for nm, src in (("wk", wk), ("wq", wq), ("wv", wv)):
                t = const.tile([128, NDT, HPC * HD], F16, tag=nm, name=nm)
                nc.sync.dma_start(
                    out=t[:], in_=src.rearrange("(dt p) f -> p dt f", p=128))
                w_sb[nm] = t
            cb = const.tile([128, 2, 2, HPC], F32, tag="cb", name="cb")
            nc.sync.dma_start(out=cb[:], in_=cvals[:])
            eb_sb = const.tile([128, 2, HPC, EBW], BF16, tag="eb", name="eb")
            nc.sync.dma_start(out=eb_sb[:],
                              in_=ebt.rearrange("m h p w -> p m h w"))
            if use_mask:
                mk = const.tile([128, NKB], F32, tag="mk", name="mk")
                nc.sync.dma_start(out=mk[:], in_=mvals[:])

            # Q^T/K^T per pair: [128(2 heads x 64d), S] bf16
            qt = [qkt.tile([128, S], F16, tag=f"qt{p}", name=f"qt{p}")
                  for p in range(2)]
            kt = [qkt.tile([128, S], F16, tag=f"kt{p}", name=f"kt{p}")
                  for p in range(2)]
            # V_ext: [128(k in block), head, kblock, 65(1|d)]
            vx = qkt.tile([128, HPC, NKB, HD + 1], BF16, tag="vx", name="vx")
            nc.sync.dma_start(
                out=vx[:, :, :, 0:1],
                in_=vones.rearrange("p (h k one) -> p h k one", h=HPC, one=1))

            for _rep in range(reps):
              # psum pools ([*,512] f32 = 1 bank each):
              # sps 4 + ctx 2 + pp 2 = 8 banks
              with tc.tile_pool(name="pp", bufs=2, space="PSUM") as pp, \
                   tc.tile_pool(name="spsp", bufs=4, space="PSUM") as spsp, \
                   tc.tile_pool(name="ctxp", bufs=2, space="PSUM") as ctxp:

                def stream_chunk(src, tag, c2):
                    """DMA one [dm, 1024]-column chunk as 8 dt tiles."""
                    tiles = []
                    for dt in range(NDT):
                        t = xs.tile([128, CW], F16, tag=f"x{dt}",
                                    name=f"x{tag}")
                        nc.sync.dma_start(
                            out=t[:],
                            in_=src[dt * 128:(dt + 1) * 128,
                                    c2 * CW:(c2 + 1) * CW])
                        tiles.append(t)
                    return tiles

                def qk_proj(wname, dst, xt, c2, drain):
                    """(X W)^T for one 1024-col chunk; 512-col psum passes."""
                    for fb in range(2):
                        for sc in range(2):
                            ps = pp.tile([128, 512], F32, tag="pp", name="pj")
                            for dt in range(NDT):
                                nc.tensor.matmul(
                                    ps[:],
                                    lhsT=w_sb[wname][:, dt,
                                                     fb * 128:(fb + 1) * 128],
                                    rhs=xt[dt][:, sc * 512:(sc + 1) * 512],
                                    start=(dt == 0), stop=(dt == NDT - 1))
                            drain(out=dst[fb][:, c2 * CW + sc * 512:
                                              c2 * CW + (sc + 1) * 512],
                                  in_=ps[:])

                def v_proj(xt, c2, drain):
                    """V[s,d] for 8 s-blocks (cols 0-255 of a 1-bank tile)."""
                    for sb in range(8):
                        ps = pp.tile([128, 512], F32, tag="pp", name="pv")
                        for dt in range(NDT):
                            nc.tensor.matmul(
                                ps[:, 0:256],
                                lhsT=xt[dt][:, sb * 128:(sb + 1) * 128],
                                rhs=w_sb["wv"][:, dt, :],
                                start=(dt == 0), stop=(dt == NDT - 1))
                        drain(out=vx[:, :, c2 * 8 + sb, 1:HD + 1],
                              in_=ps[:, 0:256].rearrange("p (h d) -> p h d",
                                                         h=HPC))

                def attn_scores(h, q2, kb):
                    """scores + exp + region fixes -> es tile (SBUF bf16)."""
                    pr, hl = h // 2, h % 2
                    sps = spsp.tile([128, QW], F32, tag="s", name="s")
                    qsl = qt[pr][hl * 64:(hl + 1) * 64,
                                 q2 * QW:(q2 + 1) * QW]
                    ksl = kt[pr][hl * 64:(hl + 1) * 64,
                                 kb * 128:(kb + 1) * 128]
                    nc.tensor.matmul(sps[:], lhsT=ksl, rhs=qsl,
                                     start=True, stop=True)
                    if use_mask:
                        nc.vector.tensor_scalar_add(
                            sps[:], sps[:], mk[:, kb:kb + 1])
                    mi = _maj_side(kb, q2)
                    es = esp.tile([128, QW], BF16, tag="es", name="es")
                    nc.scalar.activation(
                        out=es[:], in_=sps[:],
                        func=mybir.ActivationFunctionType.Exp,
                        bias=cb[:, 0, mi, h:h + 1], scale=1.0)
                    # band fix + minority saturated side, both on DVE
                    a, b = _band_bounds(kb)
                    qlo = q2 * QW
                    bs, be = max(qlo, a), min(qlo + QW, b)
                    if bs < be:
                        w0 = bs - (kb - 1) * 128
                        nc.vector.tensor_mul(
                            es[:, bs - qlo:be - qlo],
                            es[:, bs - qlo:be - qlo],
                            eb_sb[:, mi, h, w0:w0 + (be - bs)])
                    if mi == 0:
                        ms, me = max(qlo, b), qlo + QW
                    else:
                        ms, me = qlo, min(qlo + QW, a)
                    if ms < me:
                        nc.vector.tensor_scalar_mul(
                            es[:, ms - qlo:me - qlo],
                            es[:, ms - qlo:me - qlo],
                            cb[:, 1, mi, h:h + 1])
                    return es

                def attn_ctx(ctx, h, kb, es):
                    nc.tensor.matmul(
                        ctx[:], lhsT=vx[:, h, kb, :], rhs=es[:],
                        start=(kb == 0), stop=(kb == NKB - 1))

                def attn_tail(ctx, h, q2):
                    # evacuate psum; row 0 = softmax denominators
                    # (normalization happens host-side during unsharding)
                    stg = stgp.tile([HD + 1, QW], F32, tag="stg", name="stg")
                    nc.vector.tensor_copy(out=stg[:], in_=ctx[:])
                    nc.sync.dma_start(
                        out=outp[h, :, q2 * QW:(q2 + 1) * QW],
                        in_=stg[:])

                def pair_block(pr, q2, kbs, ctxs=None, held=None,
                               scores_only=False):
                    """One (head-pair, q chunk) span of k blocks.

                    The two heads' score matmuls are emitted back-to-back:
                    their contraction rows are disjoint 64-row groups, so
                    the PE runs them concurrently (row tiling).
                    """
                    h0, h1 = 2 * pr, 2 * pr + 1
                    for kb in kbs:
                        for i, h in enumerate((h0, h1)):
                            es = held.pop((h, kb), None) if held else None
                            if es is None and not scores_only:
                                es = attn_scores(h, q2, kb)
                            elif es is None:
                                held[(h, kb)] = attn_scores(h, q2, kb)
                                continue
                            attn_ctx(ctxs[i], h, kb, es)

                def pair_tail(ctxs, pr, q2):
                    attn_tail(ctxs[0], 2 * pr, q2)
                    attn_tail(ctxs[1], 2 * pr + 1, q2)

                def ctx_pair():
                    return [ctxp.tile([HD + 1, QW], F32, tag="ctx",
                                      name="ctx") for _ in range(2)]

                # ---- streaming + overlapped schedule ----
                dve_drain = nc.vector.tensor_copy
                act_drain = nc.scalar.copy

                xk0 = stream_chunk(xk, "k", 0)
                qk_proj("wk", kt, xk0, 0, act_drain)     # ACT idle early
                xq0 = stream_chunk(xq, "q", 0)
                qk_proj("wq", qt, xq0, 0, act_drain)
                xv0 = stream_chunk(xv, "v", 0)
                v_proj(xv0, 0, act_drain)

                held = {}
                # pr0/q2=0: first-half k blocks (kt/vx blocks 0-7 ready)
                ctxA = ctx_pair()
                pair_block(0, 0, range(8), ctxA)
                # pr0/q2=1: scores only; es held until vx 8-15 lands
                pair_block(0, 1, range(8), held=held, scores_only=True)

                # second halves of the K and V streams
                xk1 = stream_chunk(xk, "k", 1)
                qk_proj("wk", kt, xk1, 1, dve_drain)
                xv1 = stream_chunk(xv, "v", 1)
                v_proj(xv1, 1, dve_drain)

                # finish pr0/q2=0; then pr0/q2=1 reusing held es
                pair_block(0, 0, range(8, NKB), ctxA)
                pair_tail(ctxA, 0, 0)
                ctxB = ctx_pair()
                pair_block(0, 1, range(NKB), ctxB, held=held)
                pair_tail(ctxB, 0, 1)
                # pr1 q2=0,1 (qt cols 0-1023 already resident)
                for q2 in (0, 1):
                    ctxs = ctx_pair()
                    pair_block(1, q2, range(NKB), ctxs)
                    pair_tail(ctxs, 1, q2)

                # q2=2,3: needs the last input chunk
                xq1 = stream_chunk(xq, "q", 1)
                qk_proj("wq", qt, xq1, 1, dve_drain)
                for q2 in (2, 3):
                    for pr in (0, 1):
                        ctxs = ctx_pair()
                        pair_block(pr, q2, range(NKB), ctxs)
                        pair_tail(ctxs, pr, q2)

    nc.finalize()
    return nc


_PROG_CACHE = {}


def _get_program(use_mask):
    key = bool(use_mask)
    if key not in _PROG_CACHE:
        _PROG_CACHE[key] = build_program(key)
    return _PROG_CACHE[key]


def kernel(query, key, value, key_mask, Wq, Wk, Wv, bias_table):
    import ml_dtypes
    bf16 = ml_dtypes.bfloat16
    f16 = np.float16

    query = np.asarray(query, dtype=np.float32)
    key = np.asarray(key, dtype=np.float32)
    value = np.asarray(value, dtype=np.float32)
    key_mask = np.asarray(key_mask, dtype=np.float32)
    Wq = np.asarray(Wq, dtype=np.float32)
    Wk = np.asarray(Wk, dtype=np.float32)
    Wv = np.asarray(Wv, dtype=np.float32)
    bias_table = np.asarray(bias_table, dtype=np.float32)

    use_mask = not np.all(key_mask == 1.0)
    nc = _get_program(use_mask)

    buckets = _rel_buckets()  # [2S-1] for rel = k-q in [-(S-1), S-1]
    g = bias_table[buckets]   # [2S-1, H] bias as function of rel
    in_maps = []
    for core in range(NCORES):
        b, hg = core // 4, core % 4
        hsl = slice(hg * HPC * HD, (hg + 1) * HPC * HD)
        heads = np.arange(hg * HPC, (hg + 1) * HPC)
        c31 = bias_table[31, heads]  # rel >= +128
        c15 = bias_table[15, heads]  # rel <= -128
        cmaj = np.stack([c31, c15])               # [side, h]
        cmin = np.stack([c15, c31])
        # -32 keeps the unnormalized exps in a sane fp32 range (softmax is
        # shift-invariant; numerator and denominator scale together)
        cv = np.stack([cmaj - 32.0, np.exp(cmin - cmaj)]).astype(np.float32)
        # band tables: ebt[side, h, p, w] = exp(g_h(p - w + 128) - cmaj)
        p = np.arange(128)[:, None]
        w = np.arange(EBW)[None, :]
        rel = p - w + 128                          # in (-256, 256)
        gh = g[rel + (S - 1)][:, :, heads]         # [128, EBW, HPC]
        ebt_np = np.empty((2, HPC, 128, EBW), np.float32)
        for mi in range(2):
            ebt_np[mi] = np.exp(
                gh - cmaj[mi][None, None, :]).transpose(2, 0, 1)
        im = {
            "xv": np.ascontiguousarray(value[b].T).astype(f16),
            "xq": np.ascontiguousarray(query[b].T).astype(f16),
            "xk": np.ascontiguousarray(key[b].T).astype(f16),
            "wq": np.ascontiguousarray(Wq[:, hsl]).astype(f16),
            "wk": np.ascontiguousarray(Wk[:, hsl]).astype(f16),
            "wv": np.ascontiguousarray(Wv[:, hsl]).astype(f16),
            "ebt": ebt_np.astype(bf16),
            "cvals": np.broadcast_to(cv, (128,) + cv.shape).copy(),
            "vones": np.ones((128, HPC * NKB), bf16),
        }
        if use_mask:
            madd = (-1e4 * (1.0 - key_mask[b])).astype(np.float32)
            im["mvals"] = np.ascontiguousarray(madd.reshape(NKB, 128).T)
        in_maps.append(im)

    res = run_bass_kernel_spmd(nc, in_maps, core_ids=list(range(NCORES)))
    out = np.empty((B, S, H * HD), np.float32)
    for core in range(NCORES):
        b, hg = core // 4, core % 4
        o = res.results[core]["out"]  # [HPC, HD+1, S]; row 0 = denominators
        for h in range(HPC):
            out[b, :, (hg * HPC + h) * HD:(hg * HPC + h + 1) * HD] = \
                (o[h, 1:] / o[h, 0:1]).T
    return out
